# revision 2
# baseline (speedup 1.0000x reference)
"""Bass/Trainium2 kernel for nn_MOEFeedForward (8-expert top-2 MoE + shared expert).

Sharding: expert-parallel with token routing. The host computes the gate
(softmax + top-2, tiny: 2048x8) and dispatches each token to its two experts'
cores. Core c runs expert c's SwiGLU FFN over its gathered tokens (capacity-
padded) and also a 1/8 HID slice of the shared expert over all tokens. The
host scatter-adds the weighted expert outputs and sums the shared partials.

This does 4x less tensor-engine work than computing every expert densely:
each expert sees only its ~512 routed tokens instead of all 2048.

All operands are pre-transposed on the host so the device does no transposes
and no gate math: pure matmul + silu/mult pipelines.

Self-contained: hardcodes shapes from the problem spec.
"""
import sys

sys.path.insert(0, "/opt/trn_rl_repo")

from contextlib import ExitStack

import numpy as np
from ml_dtypes import bfloat16

import concourse.bass as bass
import concourse.tile as tile
from concourse import mybir
from concourse.bass_utils import run_bass_kernel_spmd
from concourse.vector_clock import ScopedClock

DIM = 768
HID = 2048
E = 8
T = 2048
N_CORES = 8
SH = HID // N_CORES  # shared-expert HID slice per core
DC = DIM // 128      # 6 d-chunks
HC = HID // 128      # 16 hid-chunks
SC = SH // 128       # 2 shared hid-chunks
TT = T // 128        # 16 token tiles

F32 = mybir.dt.float32
BF16 = mybir.dt.bfloat16

AF = mybir.ActivationFunctionType
OP = mybir.AluOpType


# ---------------------------------------------------------------------------
# Walrus in this container rejects CTRL instructions (NoOp/Drain) carrying
# more than one sem wait. TileContext's tail drain carries one wait per
# outstanding semaphore. Replace it with a chain of SP nops (one wait each)
# followed by a bare drain.
def _patched_drain_and_barrier(self, tick_clock, wait_clock):
    import bass_rust

    nop_inst = self.nc.sync.nop(nofuse=True, hint="pre_drain_wait_funnel")
    wait_clock.add_sem_waits(
        nop_inst.ins, ScopedClock({None: tick_clock.global_clock})
    )
    si = nop_inst.ins.sync_info
    waits = list(si.on_wait) if si else []
    if len(waits) > 1:
        nop_inst.ins.sync_info.on_wait = waits[:1]
        for w in waits[1:]:
            extra = self.nc.sync.nop(nofuse=True, hint="pre_drain_wait_funnel")
            extra.ins.sync_info = bass_rust.SyncInfo(on_wait=[w], on_update=[])
    self.nc.sync.drain()

    self.nc.all_engine_barrier()
    assert self.sems is not None
    popped = self.nc._tile_sem_poison_stack.pop()
    assert popped is self._sem_poison
    self.nc.clear_and_free_semaphores(list(self.sems.allocated().values()))
    self.nc.all_engine_barrier()


tile.TileContext._drain_and_barrier = _patched_drain_and_barrier


def _split_multi_waits(nc, max_waits=1):
    """This walrus build allows at most one sem wait per instruction. Hoist
    extra waits onto same-engine nops inserted immediately before."""
    import bass_rust

    n_split = 0
    for f in nc.m.functions:
        for bb in f.blocks:
            il = bb.instructions
            i = 0
            while i < len(il):
                inst = il[i]
                si = inst.sync_info
                if si is None or len(si.on_wait) <= max_waits:
                    i += 1
                    continue
                waits = list(si.on_wait)
                si.on_wait = waits[:max_waits]
                for k, w in enumerate(waits[max_waits:]):
                    nop = mybir.InstNoOp(
                        name=f"{inst.name}-wsplit{k}", ins=[], outs=[]
                    )
                    nop.engine = inst.engine
                    nop.sync_info = bass_rust.SyncInfo(on_wait=[w], on_update=[])
                    il.insert(i, nop)
                    i += 1
                n_split += 1
                i += 1
    return n_split
# ---------------------------------------------------------------------------


def _build_kernel(C):
    """C: per-core expert token capacity (multiple of 128)."""
    CT = C // 128  # token tiles of gathered tokens

    nc = bass.Bass()
    xT_d = nc.dram_tensor("xT", [DIM, T], BF16, kind="ExternalInput")
    xgT_d = nc.dram_tensor("xgT", [DIM, C], BF16, kind="ExternalInput")
    wg_d = nc.dram_tensor("wg", [128, CT], F32, kind="ExternalInput")
    w1T_d = nc.dram_tensor("w1T", [DIM, HID], BF16, kind="ExternalInput")
    w3T_d = nc.dram_tensor("w3T", [DIM, HID], BF16, kind="ExternalInput")
    w2T_d = nc.dram_tensor("w2T", [HID, DIM], BF16, kind="ExternalInput")
    s1T_d = nc.dram_tensor("s1T", [DIM, SH], BF16, kind="ExternalInput")
    s3T_d = nc.dram_tensor("s3T", [DIM, SH], BF16, kind="ExternalInput")
    s2T_d = nc.dram_tensor("s2T", [SH, DIM], BF16, kind="ExternalInput")
    yg_d = nc.dram_tensor("yg", [C, DIM], BF16, kind="ExternalOutput")
    ys_d = nc.dram_tensor("ys", [T, DIM], BF16, kind="ExternalOutput")

    with tile.TileContext(nc) as tc, ExitStack() as ctx:
        persist = ctx.enter_context(tc.tile_pool(name="persist", bufs=1))
        silu_p = ctx.enter_context(tc.tile_pool(name="silu", bufs=3))
        yo_p = ctx.enter_context(tc.tile_pool(name="yo", bufs=3))
        h_ps = ctx.enter_context(tc.tile_pool(name="h_ps", bufs=4, space="PSUM"))
        y_ps = ctx.enter_context(tc.tile_pool(name="y_ps", bufs=4, space="PSUM"))

        # Persistent SBUF tensors (pre-transposed bf16 operands)
        xT = persist.tile([128, DC, T], BF16, tag="xT")      # [d, t] all tokens
        xgT = persist.tile([128, DC, C], BF16, tag="xgT")    # [d, t] routed tokens
        w1T = persist.tile([128, DC, HID], BF16, tag="w1T")  # [d, hid]
        w3T = persist.tile([128, DC, HID], BF16, tag="w3T")
        w2T = persist.tile([128, HC, DIM], BF16, tag="w2T")  # [hid, d]
        s1T = persist.tile([128, DC, SH], BF16, tag="s1T")
        s3T = persist.tile([128, DC, SH], BF16, tag="s3T")
        s2T = persist.tile([128, SC, DIM], BF16, tag="s2T")
        wg = persist.tile([128, CT], F32, tag="wg")          # combine weight/token
        hT = persist.tile([128, HC, C], BF16, tag="hT")      # expert hidden
        shT = persist.tile([128, SC, T], BF16, tag="shT")    # shared hidden

        # --- input DMAs, in consumption order
        def load_chunks(src_d, dst, nchunks):
            for c in range(nchunks):
                nc.sync.dma_start(dst[:, c, :], src_d[c * 128:(c + 1) * 128, :])

        load_chunks(xT_d, xT, DC)
        load_chunks(s1T_d, s1T, DC)
        load_chunks(s3T_d, s3T, DC)
        load_chunks(xgT_d, xgT, DC)
        load_chunks(w1T_d, w1T, DC)
        load_chunks(w3T_d, w3T, DC)
        load_chunks(w2T_d, w2T, HC)
        load_chunks(s2T_d, s2T, SC)
        nc.sync.dma_start(wg[:], wg_d[:])

        def swiglu_h(a1T, a3T, xsrc, hout, hb, fsl, fw):
            p1 = h_ps.tile([128, 512], F32, tag="hps")
            for dc in range(DC):
                nc.tensor.matmul(
                    p1[:, :fw], a1T[:, dc, hb * 128:(hb + 1) * 128],
                    xsrc[:, dc, fsl],
                    start=(dc == 0), stop=(dc == DC - 1),
                )
            p3 = h_ps.tile([128, 512], F32, tag="hps")
            for dc in range(DC):
                nc.tensor.matmul(
                    p3[:, :fw], a3T[:, dc, hb * 128:(hb + 1) * 128],
                    xsrc[:, dc, fsl],
                    start=(dc == 0), stop=(dc == DC - 1),
                )
            sl = silu_p.tile([128, 512], BF16, tag="silu")
            nc.scalar.activation(sl[:, :fw], p1[:, :fw], AF.Silu)
            nc.vector.tensor_tensor(
                hout[:, hb, fsl], sl[:, :fw], p3[:, :fw], op=OP.mult
            )

        # --- shared-expert h-stage: shT[hid_sl, t] over all tokens
        for hb in range(SC):
            for fb in range(T // 512):
                swiglu_h(s1T, s3T, xT, shT, hb,
                         slice(fb * 512, (fb + 1) * 512), 512)

        # --- expert h-stage: hT[hid, t] over routed tokens
        fblocks = []
        fo = 0
        while fo < C:
            fw = min(512, C - fo)
            fblocks.append((slice(fo, fo + fw), fw))
            fo += fw
        for hb in range(HC):
            for fsl, fw in fblocks:
                swiglu_h(w1T, w3T, xgT, hT, hb, fsl, fw)

        # --- expert mm2: yg[t, d] = (hT.T @ w2T) * wg, per 128-token tile
        for tb in range(CT):
            tsl = slice(tb * 128, (tb + 1) * 128)
            yo = yo_p.tile([128, DIM], BF16, tag="yo")
            for dh in range(2):
                dsl = slice(dh * 384, (dh + 1) * 384)
                pe = y_ps.tile([128, 384], F32, tag="y")
                for hb in range(HC):
                    nc.tensor.matmul(
                        pe[:], hT[:, hb, tsl], w2T[:, hb, dsl],
                        start=(hb == 0), stop=(hb == HC - 1),
                    )
                nc.vector.tensor_scalar(
                    yo[:, dsl], pe[:], wg[:, tb:tb + 1], None, op0=OP.mult
                )
            nc.sync.dma_start(yg_d[tb * 128:(tb + 1) * 128, :], yo[:])

        # --- shared mm2: ys[t, d] = shT.T @ s2T, per 128-token tile
        for tb in range(TT):
            tsl = slice(tb * 128, (tb + 1) * 128)
            yso = yo_p.tile([128, DIM], BF16, tag="yso")
            for dh in range(2):
                dsl = slice(dh * 384, (dh + 1) * 384)
                ps = y_ps.tile([128, 384], F32, tag="y")
                for sc in range(SC):
                    nc.tensor.matmul(
                        ps[:], shT[:, sc, tsl], s2T[:, sc, dsl],
                        start=(sc == 0), stop=(sc == SC - 1),
                    )
                nc.scalar.copy(yso[:, dsl], ps[:])
            nc.sync.dma_start(ys_d[tb * 128:(tb + 1) * 128, :], yso[:])

    _split_multi_waits(nc)
    try:
        _CACHE["makespan_ns"] = max(e[2] for e in tc._perfetto_entries)
    except Exception:
        _CACHE["makespan_ns"] = None
    return nc


_CACHE = {}


def _route(x2, gate_w):
    """Host gate: softmax + top-2 + normalize. Returns per-expert token
    lists and combine weights."""
    logits = x2 @ gate_w.T                       # [T, E] f32
    logits -= logits.max(1, keepdims=True)
    p = np.exp(logits)
    p /= p.sum(1, keepdims=True)
    tix = np.arange(p.shape[0])
    i1 = p.argmax(1)
    pm = p.copy()
    pm[tix, i1] = -np.inf
    i2 = pm.argmax(1)
    p1 = p[tix, i1]
    p2 = p[tix, i2]
    s = p1 + p2 + 1e-20
    w1n = p1 / s
    w2n = p2 / s
    sels, wgts = [], []
    for c in range(E):
        sel = np.flatnonzero((i1 == c) | (i2 == c))
        w_sel = np.where(i1[sel] == c, w1n[sel], w2n[sel]).astype(np.float32)
        sels.append(sel)
        wgts.append(w_sel)
    return sels, wgts


def kernel(x, gate_w, w1, w2, w3, ws1, ws2, ws3):
    x = np.asarray(x, dtype=np.float32)
    gate_w = np.ascontiguousarray(np.asarray(gate_w, dtype=np.float32))
    w1 = np.asarray(w1, dtype=np.float32)
    w2 = np.asarray(w2, dtype=np.float32)
    w3 = np.asarray(w3, dtype=np.float32)
    ws1 = np.asarray(ws1, dtype=np.float32)
    ws2 = np.asarray(ws2, dtype=np.float32)
    ws3 = np.asarray(ws3, dtype=np.float32)

    B, S, D = x.shape
    x2 = np.ascontiguousarray(x.reshape(-1, D))

    sels, wgts = _route(x2, gate_w)
    maxn = max(len(s) for s in sels)
    C = max(640, -(-maxn // 128) * 128)

    if _CACHE.get("C") != C:
        _CACHE["C"] = C
        _CACHE["nc"] = _build_kernel(C)
    nc = _CACHE["nc"]
    CT = C // 128

    xT_full = np.ascontiguousarray(x2.T.astype(bfloat16))
    in_maps = []
    for c in range(N_CORES):
        sel = sels[c]
        n = len(sel)
        xgT = np.zeros((D, C), dtype=bfloat16)
        xgT[:, :n] = x2[sel].T
        wg = np.zeros((C,), dtype=np.float32)
        wg[:n] = wgts[c]
        sh = slice(c * SH, (c + 1) * SH)
        in_maps.append({
            "xT": xT_full,
            "xgT": xgT,
            "wg": np.ascontiguousarray(wg.reshape(CT, 128).T),
            "w1T": np.ascontiguousarray(w1[c].T.astype(bfloat16)),
            "w3T": np.ascontiguousarray(w3[c].T.astype(bfloat16)),
            "w2T": np.ascontiguousarray(w2[c].T.astype(bfloat16)),
            "s1T": np.ascontiguousarray(ws1[sh].T.astype(bfloat16)),
            "s3T": np.ascontiguousarray(ws3[sh].T.astype(bfloat16)),
            "s2T": np.ascontiguousarray(ws2[:, sh].T.astype(bfloat16)),
        })

    _CACHE["last_in_maps"] = in_maps
    res = run_bass_kernel_spmd(nc, in_maps, list(range(N_CORES)))
    y = np.zeros((T, D), dtype=np.float32)
    for c in range(N_CORES):
        sel = sels[c]
        y[sel] += np.asarray(res.results[c]["yg"][:len(sel)], dtype=np.float32)
        y += np.asarray(res.results[c]["ys"], dtype=np.float32)
    return y.reshape(B, S, D)


# revision 20
# speedup vs baseline: 3.1640x; 3.1640x over previous
"""Bass/Trainium2 kernel for nn_MOEFeedForward (8-expert top-2 MoE + shared expert).

Sharding: expert-parallel with token routing. The host computes the gate
(softmax + top-2, tiny: 2048x8) and dispatches each token to its two experts'
cores. Core c runs expert c's SwiGLU FFN over its gathered tokens (capacity
C=512) plus a 1/8 HID slice of the shared expert over all tokens. Load
imbalance beyond C is absorbed by a fixed 128-token "overflow" slot per core:
an overloaded expert's excess tokens are computed across 4 cores, each doing
one quarter of HID (the FFN is additive over hidden units), so every core
runs an identical (SPMD) program at the balanced-load cycle count.

The host scatter-adds the weighted expert outputs and sums the shared/quarter
partials. This does ~4x less tensor-engine work than computing every expert
densely over all tokens.

All operands are pre-transposed AND pre-tiled to [128, ...] partition-major
layouts on the host, so every input loads with one large DMA and the device
does no transposes and no gate math: pure matmul + silu/mult pipelines.

Self-contained: hardcodes shapes from the problem spec.
"""
import sys

sys.path.insert(0, "/opt/trn_rl_repo")

from contextlib import ExitStack

import numpy as np
from ml_dtypes import bfloat16

import concourse.bass as bass
import concourse.tile as tile
from concourse import mybir
from concourse.bass_utils import run_bass_kernel_spmd
from concourse.vector_clock import ScopedClock

DIM = 768
HID = 2048
E = 8
T = 2048
N_CORES = 8
SH = HID // N_CORES  # shared-expert HID slice per core
QH = HID // 4        # overflow quarter-HID slice
DC = DIM // 128      # 6 d-chunks
HC = HID // 128      # 16 hid-chunks
SC = SH // 128       # 2 shared hid-chunks
QC = QH // 128       # 4 overflow hid-chunks
TT = T // 128        # 16 token tiles
CO = 128             # overflow token capacity per slot

F32 = mybir.dt.float32
BF16 = mybir.dt.bfloat16

AF = mybir.ActivationFunctionType
OP = mybir.AluOpType


# ---------------------------------------------------------------------------
# Walrus in this container rejects CTRL instructions (NoOp/Drain) carrying
# more than one sem wait. TileContext's tail drain carries one wait per
# outstanding semaphore. Replace it with a chain of SP nops (one wait each)
# followed by a bare drain.
def _patched_drain_and_barrier(self, tick_clock, wait_clock):
    import bass_rust

    nop_inst = self.nc.sync.nop(nofuse=True, hint="pre_drain_wait_funnel")
    wait_clock.add_sem_waits(
        nop_inst.ins, ScopedClock({None: tick_clock.global_clock})
    )
    si = nop_inst.ins.sync_info
    waits = list(si.on_wait) if si else []
    if len(waits) > 1:
        nop_inst.ins.sync_info.on_wait = waits[:1]
        for w in waits[1:]:
            extra = self.nc.sync.nop(nofuse=True, hint="pre_drain_wait_funnel")
            extra.ins.sync_info = bass_rust.SyncInfo(on_wait=[w], on_update=[])
    self.nc.sync.drain()

    self.nc.all_engine_barrier()
    assert self.sems is not None
    popped = self.nc._tile_sem_poison_stack.pop()
    assert popped is self._sem_poison
    self.nc.clear_and_free_semaphores(list(self.sems.allocated().values()))
    self.nc.all_engine_barrier()


tile.TileContext._drain_and_barrier = _patched_drain_and_barrier


def _split_multi_waits(nc, max_waits=1):
    """This walrus build allows at most one sem wait per instruction. Hoist
    extra waits onto same-engine nops inserted immediately before."""
    import bass_rust

    n_split = 0
    for f in nc.m.functions:
        for bb in f.blocks:
            il = bb.instructions
            i = 0
            while i < len(il):
                inst = il[i]
                si = inst.sync_info
                if si is None or len(si.on_wait) <= max_waits:
                    i += 1
                    continue
                waits = list(si.on_wait)
                si.on_wait = waits[:max_waits]
                for k, w in enumerate(waits[max_waits:]):
                    nop = mybir.InstNoOp(
                        name=f"{inst.name}-wsplit{k}", ins=[], outs=[]
                    )
                    nop.engine = inst.engine
                    nop.sync_info = bass_rust.SyncInfo(on_wait=[w], on_update=[])
                    il.insert(i, nop)
                    i += 1
                n_split += 1
                i += 1
    return n_split
# ---------------------------------------------------------------------------


def _build_kernel(C, reps=1):
    """C: per-core expert token capacity (multiple of 128).

    reps>1 repeats the whole compute phase (DMAs included once) — used by
    test.py for slope-based timing that cancels per-call dispatch overhead.
    """
    CT = C // 128  # token tiles of gathered tokens

    nc = bass.Bass()
    # All inputs arrive pre-tiled by the host into [128, ...] layouts so each
    # tensor loads with one or two large DMAs (per-DMA issue overhead is
    # ~0.8us, and the first matmuls otherwise starve).
    x4_d = nc.dram_tensor("x4", [4 * 128, DC, 512], BF16, kind="ExternalInput")
    xg_d = nc.dram_tensor("xg", [128, DC * C], BF16, kind="ExternalInput")
    wg_d = nc.dram_tensor("wg", [128, CT], F32, kind="ExternalInput")
    w13_d = nc.dram_tensor("w13", [2 * 128, DC * 2048], BF16, kind="ExternalInput")
    w2_d = nc.dram_tensor("w2", [128, HC * DIM], BF16, kind="ExternalInput")
    s13_d = nc.dram_tensor("s13", [128, DC * 2 * SH], BF16, kind="ExternalInput")
    s2_d = nc.dram_tensor("s2", [128, SC * DIM], BF16, kind="ExternalInput")
    xo_d = nc.dram_tensor("xo", [128, DC * CO], BF16, kind="ExternalInput")
    wgo_d = nc.dram_tensor("wgo", [128, 1], F32, kind="ExternalInput")
    w13q_d = nc.dram_tensor("w13q", [128, DC * 2 * QH], BF16, kind="ExternalInput")
    w2q_d = nc.dram_tensor("w2q", [128, QC * DIM], BF16, kind="ExternalInput")
    yg_d = nc.dram_tensor("yg", [C, DIM], BF16, kind="ExternalOutput")
    ys_d = nc.dram_tensor("ys", [T, DIM], BF16, kind="ExternalOutput")
    yo_d = nc.dram_tensor("yo", [CO, DIM], BF16, kind="ExternalOutput")

    with tile.TileContext(nc) as tc, ExitStack() as ctx:
        persist = ctx.enter_context(tc.tile_pool(name="persist", bufs=1))
        silu_p = ctx.enter_context(tc.tile_pool(name="silu", bufs=3))
        yo_p = ctx.enter_context(tc.tile_pool(name="yo", bufs=6))
        h_ps = ctx.enter_context(tc.tile_pool(name="h_ps", bufs=4, space="PSUM"))
        y_ps = ctx.enter_context(tc.tile_pool(name="y_ps", bufs=4, space="PSUM"))

        # Persistent SBUF tensors (pre-transposed bf16 operands)
        xT4 = persist.tile([128, 4, DC, 512], BF16, tag="xT4")   # all tokens, quarter-major
        xgT = persist.tile([128, DC, C], BF16, tag="xgT")        # routed tokens
        w13T = persist.tile([128, 2, DC, 2048], BF16, tag="w13T")  # w1|w3 by hid-half
        w2T = persist.tile([128, HC, DIM], BF16, tag="w2T")      # [hid, d]
        s13T = persist.tile([128, DC, 2 * SH], BF16, tag="s13T")  # s1|s3 [d, hid_sl]
        s2T = persist.tile([128, SC, DIM], BF16, tag="s2T")
        wg = persist.tile([128, CT], F32, tag="wg")              # combine weight/token
        xoT = persist.tile([128, DC, CO], BF16, tag="xoT")       # overflow tokens
        w13qT = persist.tile([128, DC, 2 * QH], BF16, tag="w13qT")  # w1q|w3q [d, hid_q]
        w2qT = persist.tile([128, QC, DIM], BF16, tag="w2qT")
        wgo = persist.tile([128, 1], F32, tag="wgo")
        hT = persist.tile([128, HC, C], BF16, tag="hT")          # expert hidden
        shT = persist.tile([128, SC, T], BF16, tag="shT")        # shared hidden
        hoT = persist.tile([128, QC, CO], BF16, tag="hoT")       # overflow hidden

        # Issue order == consumption order; transfers drain FIFO through the
        # shared DMA engines, so x quarters land just ahead of the PE. The
        # first pieces are small so the first matmul chain starts ASAP.
        nc.sync.dma_start(s13T[:, :, 0:SH], s13_d[:, 0:DC * SH])
        nc.sync.dma_start(xT4[:, 0, :, 0:256], x4_d[0:128, :, 0:256])
        nc.sync.dma_start(s13T[:, :, SH:2 * SH], s13_d[:, DC * SH:DC * 2 * SH])
        nc.sync.dma_start(xT4[:, 0, :, 256:512], x4_d[0:128, :, 256:512])
        for q in range(1, 4):
            nc.sync.dma_start(xT4[:, q, :, :], x4_d[q * 128:(q + 1) * 128, :, :])
        nc.sync.dma_start(xgT[:], xg_d[:])
        for g in range(2):
            nc.sync.dma_start(w13T[:, g, :, :], w13_d[g * 128:(g + 1) * 128, :])
        nc.sync.dma_start(s2T[:], s2_d[:])
        nc.sync.dma_start(xoT[:], xo_d[:])
        nc.sync.dma_start(w13qT[:], w13q_d[:])
        nc.sync.dma_start(w2T[:], w2_d[:])
        nc.sync.dma_start(w2qT[:], w2q_d[:])
        nc.sync.dma_start(wg[:], wg_d[:])
        nc.sync.dma_start(wgo[:], wgo_d[:])

        def swiglu_h(lhs1, lhs3, rhs, hout, hb, fsl, fw):
            """lhs1/lhs3: dc -> [128,128] AP; rhs: dc -> [128,fw] AP."""
            p1 = h_ps.tile([128, 512], F32, tag="hps")
            for dc in range(DC):
                nc.tensor.matmul(
                    p1[:, :fw], lhs1(dc), rhs(dc),
                    start=(dc == 0), stop=(dc == DC - 1),
                )
            p3 = h_ps.tile([128, 512], F32, tag="hps")
            for dc in range(DC):
                nc.tensor.matmul(
                    p3[:, :fw], lhs3(dc), rhs(dc),
                    start=(dc == 0), stop=(dc == DC - 1),
                )
            sl = silu_p.tile([128, 512], BF16, tag="silu")
            nc.scalar.activation(sl[:, :fw], p1[:, :fw], AF.Silu)
            nc.vector.tensor_tensor(
                hout[:, hb, fsl], sl[:, :fw], p3[:, :fw], op=OP.mult
            )

        fblocks = []
        fo = 0
        while fo < C:
            fw = min(512, C - fo)
            fblocks.append((slice(fo, fo + fw), fw))
            fo += fw

        for _rep in range(reps):
            # --- shared-expert h-stage: shT[hid_sl, t] over all tokens.
            # Quarter-outer so PE consumption tracks the x4 DMA arrivals; the
            # first quarter runs as two 256-wide chains to match its split DMA.
            for q in range(4):
                subs = [(0, 256), (256, 256)] if q == 0 else [(0, 512)]
                for so, sw in subs:
                    for hb in range(SC):
                        swiglu_h(
                            lambda dc, hb=hb: s13T[:, dc, hb * 128:(hb + 1) * 128],
                            lambda dc, hb=hb: s13T[:, dc, SH + hb * 128:SH + (hb + 1) * 128],
                            lambda dc, q=q, so=so, sw=sw: xT4[:, q, dc, so:so + sw],
                            shT, hb, slice(q * 512 + so, q * 512 + so + sw), sw,
                        )

            # --- expert h-stage: hT[hid, t] over routed tokens
            for hb in range(HC):
                g, r = divmod(hb, 8)
                for fsl, fw in fblocks:
                    swiglu_h(
                        lambda dc, g=g, r=r: w13T[:, g, dc, r * 128:(r + 1) * 128],
                        lambda dc, g=g, r=r: w13T[:, g, dc, 1024 + r * 128:1024 + (r + 1) * 128],
                        lambda dc, fsl=fsl: xgT[:, dc, fsl],
                        hT, hb, fsl, fw,
                    )

            # --- overflow h-stage: quarter-HID FFN for the overflow tokens
            for hq in range(QC):
                swiglu_h(
                    lambda dc, hq=hq: w13qT[:, dc, hq * 128:(hq + 1) * 128],
                    lambda dc, hq=hq: w13qT[:, dc, QH + hq * 128:QH + (hq + 1) * 128],
                    lambda dc: xoT[:, dc, :],
                    hoT, hq, slice(0, CO), CO,
                )

            # --- shared mm2: ys[t, d] = shT.T @ s2T, per 128-token tile.
            # Runs between the h stages and expert-mm2: its matmuls only need
            # shT (done early), so they fill the PE bubble while the last
            # expert hT tiles drain through silu/mult. PSUM->bf16 copies split
            # between Act and DVE, output DMAs split between SP and Pool, so
            # no single engine gates the PE.
            for tb in range(TT):
                tsl = slice(tb * 128, (tb + 1) * 128)
                yso = yo_p.tile([128, DIM], BF16, tag="yso")
                for dh in range(2):
                    dsl = slice(dh * 384, (dh + 1) * 384)
                    ps = y_ps.tile([128, 384], F32, tag="y")
                    for sc in range(SC):
                        nc.tensor.matmul(
                            ps[:], shT[:, sc, tsl], s2T[:, sc, dsl],
                            start=(sc == 0), stop=(sc == SC - 1),
                        )
                    if dh == 0:
                        nc.scalar.copy(yso[:, dsl], ps[:])
                    else:
                        nc.vector.tensor_copy(yso[:, dsl], ps[:])
                eng = nc.sync if tb % 2 == 0 else nc.gpsimd
                eng.dma_start(ys_d[tb * 128:(tb + 1) * 128, :], yso[:])

            # --- overflow mm2: yo[t, d] = (hoT.T @ w2qT) * wgo
            yoo = yo_p.tile([128, DIM], BF16, tag="ygo")
            for dh in range(2):
                dsl = slice(dh * 384, (dh + 1) * 384)
                po = y_ps.tile([128, 384], F32, tag="y")
                for hq in range(QC):
                    nc.tensor.matmul(
                        po[:], hoT[:, hq, :], w2qT[:, hq, dsl],
                        start=(hq == 0), stop=(hq == QC - 1),
                    )
                nc.vector.tensor_scalar(
                    yoo[:, dsl], po[:], wgo[:, 0:1], None, op0=OP.mult
                )
            nc.gpsimd.dma_start(yo_d[:], yoo[:])

            # --- expert mm2: yg[t, d] = (hT.T @ w2T) * wg, per 128-token tile
            for tb in range(CT):
                tsl = slice(tb * 128, (tb + 1) * 128)
                yo = yo_p.tile([128, DIM], BF16, tag="yo")
                for dh in range(2):
                    dsl = slice(dh * 384, (dh + 1) * 384)
                    pe = y_ps.tile([128, 384], F32, tag="y")
                    for hb in range(HC):
                        nc.tensor.matmul(
                            pe[:], hT[:, hb, tsl], w2T[:, hb, dsl],
                            start=(hb == 0), stop=(hb == HC - 1),
                        )
                    nc.vector.tensor_scalar(
                        yo[:, dsl], pe[:], wg[:, tb:tb + 1], None, op0=OP.mult
                    )
                nc.sync.dma_start(yg_d[tb * 128:(tb + 1) * 128, :], yo[:])

    _split_multi_waits(nc)
    try:
        _CACHE["makespan_ns"] = max(e[2] for e in tc._perfetto_entries)
    except Exception:
        _CACHE["makespan_ns"] = None
    return nc


_CACHE = {}


def _route(x2, gate_w):
    """Host gate: softmax + top-2 + normalize. Returns per-expert token
    lists and combine weights."""
    logits = x2 @ gate_w.T                       # [T, E] f32
    logits -= logits.max(1, keepdims=True)
    p = np.exp(logits)
    p /= p.sum(1, keepdims=True)
    tix = np.arange(p.shape[0])
    i1 = p.argmax(1)
    pm = p.copy()
    pm[tix, i1] = -np.inf
    i2 = pm.argmax(1)
    p1 = p[tix, i1]
    p2 = p[tix, i2]
    s = p1 + p2 + 1e-20
    w1n = p1 / s
    w2n = p2 / s
    sels, wgts = [], []
    for c in range(E):
        sel = np.flatnonzero((i1 == c) | (i2 == c))
        w_sel = np.where(i1[sel] == c, w1n[sel], w2n[sel]).astype(np.float32)
        sels.append(sel)
        wgts.append(w_sel)
    return sels, wgts


def _tile_cols(aT, nchunks):
    # aT: [nchunks*128, M] -> [128, nchunks*M] with [p, c*M+m] = aT[c*128+p, m]
    M = aT.shape[1]
    return np.ascontiguousarray(
        aT.reshape(nchunks, 128, M).transpose(1, 0, 2).reshape(128, nchunks * M)
    )


def kernel(x, gate_w, w1, w2, w3, ws1, ws2, ws3):
    x = np.asarray(x, dtype=np.float32)
    gate_w = np.ascontiguousarray(np.asarray(gate_w, dtype=np.float32))
    w1 = np.asarray(w1, dtype=np.float32)
    w2 = np.asarray(w2, dtype=np.float32)
    w3 = np.asarray(w3, dtype=np.float32)
    ws1 = np.asarray(ws1, dtype=np.float32)
    ws2 = np.asarray(ws2, dtype=np.float32)
    ws3 = np.asarray(ws3, dtype=np.float32)

    B, S, D = x.shape
    x2 = np.ascontiguousarray(x.reshape(-1, D))

    sels, wgts = _route(x2, gate_w)

    # Capacity plan: C=512 + one 128-token quarter-HID overflow slot per core
    # handles up to 2 experts exceeding C by up to CO tokens each. Otherwise
    # fall back to a plain capacity-padded build (correct for any routing).
    C = 512
    over = [c for c in range(E) if len(sels[c]) > C]
    if len(over) > 2 or any(len(sels[c]) > C + CO for c in over):
        C = -(-max(len(s) for s in sels) // 128) * 128
        over = []

    if _CACHE.get("C") != C:
        _CACHE["C"] = C
        _CACHE["nc"] = _build_kernel(C)
    nc = _CACHE["nc"]
    CT = C // 128

    xb = x2.astype(bfloat16)
    # x4[q*128+p, dc*512+j] = x[q*512+j, dc*128+p]
    x4 = np.ascontiguousarray(
        xb.reshape(4, 512, DC, 128).transpose(0, 3, 2, 1).reshape(4 * 128, DC, 512)
    )

    # overflow slot assignment: expert over[k] -> cores 4k..4k+3, quarter q
    ovf_sel = {}
    for k, e in enumerate(over):
        ovf_sel[e] = sels[e][C:]

    in_maps = []
    for c in range(N_CORES):
        sel = sels[c][:C]
        n = len(sel)
        xgT = np.zeros((D, C), dtype=bfloat16)
        xgT[:, :n] = x2[sel].T
        wgv = np.zeros((C,), dtype=np.float32)
        wgv[:n] = wgts[c][:n]
        sh = slice(c * SH, (c + 1) * SH)
        # w13[g*128+p, dc*2048+j]: j<1024 -> w1T[dc*128+p, g*1024+j], else w3T
        w1r = w1[c].T.astype(bfloat16).reshape(DC, 128, 2, 1024).transpose(2, 1, 0, 3)
        w3r = w3[c].T.astype(bfloat16).reshape(DC, 128, 2, 1024).transpose(2, 1, 0, 3)
        w13 = np.ascontiguousarray(
            np.concatenate([w1r, w3r], axis=3).reshape(2 * 128, DC * 2048)
        )
        # s13 halves: cols [0:DC*SH] = s1 (dc-major), [DC*SH:] = s3 — matches
        # the two split DMAs whose dst is s13T[:, :, 0:SH] / [:, :, SH:2*SH].
        s1r = ws1[sh].T.astype(bfloat16).reshape(DC, 128, SH).transpose(1, 0, 2)
        s3r = ws3[sh].T.astype(bfloat16).reshape(DC, 128, SH).transpose(1, 0, 2)
        s13 = np.ascontiguousarray(np.concatenate(
            [s1r.reshape(128, DC * SH), s3r.reshape(128, DC * SH)], axis=1
        ))

        # overflow slot inputs for this core
        xoT = np.zeros((D, CO), dtype=bfloat16)
        wgo = np.zeros((CO,), dtype=np.float32)
        w13q = np.zeros((D, 2 * QH), dtype=bfloat16)
        w2q = np.zeros((QH, DIM), dtype=bfloat16)
        k, q = divmod(c, 4)
        if k < len(over):
            e = over[k]
            osel = ovf_sel[e]
            no = len(osel)
            xoT[:, :no] = x2[osel].T
            wgo[:no] = wgts[e][C:]
            h0 = q * QH
            w13q[:, :QH] = w1[e].T[:, h0:h0 + QH].astype(bfloat16)
            w13q[:, QH:] = w3[e].T[:, h0:h0 + QH].astype(bfloat16)
            w2q[:, :] = w2[e].T[h0:h0 + QH, :].astype(bfloat16)

        in_maps.append({
            "x4": x4,
            "xg": _tile_cols(xgT, DC),
            "wg": np.ascontiguousarray(wgv.reshape(CT, 128).T),
            "w13": w13,
            "w2": _tile_cols(w2[c].T.astype(bfloat16), HC),
            "s13": s13,
            "s2": _tile_cols(ws2[:, sh].T.astype(bfloat16), SC),
            "xo": _tile_cols(xoT, DC),
            "wgo": np.ascontiguousarray(wgo.reshape(1, 128).T),
            "w13q": _tile_cols(w13q, DC),
            "w2q": _tile_cols(w2q, QC),
        })

    _CACHE["last_in_maps"] = in_maps
    res = run_bass_kernel_spmd(nc, in_maps, list(range(N_CORES)))
    y = np.zeros((T, D), dtype=np.float32)
    for c in range(N_CORES):
        sel = sels[c][:C]
        y[sel] += np.asarray(res.results[c]["yg"][:len(sel)], dtype=np.float32)
        y += np.asarray(res.results[c]["ys"], dtype=np.float32)
        k = c // 4
        if k < len(over):
            osel = ovf_sel[over[k]]
            y[osel] += np.asarray(res.results[c]["yo"][:len(osel)], dtype=np.float32)
    return y.reshape(B, S, D)


# revision 31
# speedup vs baseline: 3.2141x; 1.0158x over previous
"""Bass/Trainium2 kernel for nn_MOEFeedForward (8-expert top-2 MoE + shared expert).

Sharding: expert-parallel with token routing. The host computes the gate
(softmax + top-2, tiny: 2048x8) and dispatches each token to its two experts'
cores. Core c runs expert c's SwiGLU FFN over its gathered tokens (capacity
C=512) plus a 1/8 HID slice of the shared expert over all tokens. Load
imbalance beyond C is absorbed by a fixed 128-token "overflow" slot per core:
an overloaded expert's excess tokens are computed across 4 cores, each doing
one quarter of HID (the FFN is additive over hidden units), so every core
runs an identical (SPMD) program at the balanced-load cycle count.

The host scatter-adds the weighted expert outputs and sums the shared/quarter
partials. This does ~4x less tensor-engine work than computing every expert
densely over all tokens.

All operands are pre-transposed AND pre-tiled to [128, ...] partition-major
layouts on the host, so every input loads with one large DMA and the device
does no transposes and no gate math: pure matmul + silu/mult pipelines.

Self-contained: hardcodes shapes from the problem spec.
"""
import sys

sys.path.insert(0, "/opt/trn_rl_repo")

from contextlib import ExitStack

import numpy as np
from ml_dtypes import bfloat16

import concourse.bass as bass
import concourse.tile as tile
from concourse import mybir
from concourse.bass_utils import run_bass_kernel_spmd
from concourse.masks import make_identity
from concourse.vector_clock import ScopedClock

DIM = 768
HID = 2048
E = 8
T = 2048
N_CORES = 8
SH = HID // N_CORES  # shared-expert HID slice per core
QH = HID // 4        # overflow quarter-HID slice
DC = DIM // 128      # 6 d-chunks
HC = HID // 128      # 16 hid-chunks
SC = SH // 128       # 2 shared hid-chunks
QC = QH // 128       # 4 overflow hid-chunks
TT = T // 128        # 16 token tiles
CO = 64              # overflow token capacity per slot

F32 = mybir.dt.float32
BF16 = mybir.dt.bfloat16

AF = mybir.ActivationFunctionType
OP = mybir.AluOpType


# ---------------------------------------------------------------------------
# Walrus in this container rejects CTRL instructions (NoOp/Drain) carrying
# more than one sem wait. TileContext's tail drain carries one wait per
# outstanding semaphore. Replace it with a chain of SP nops (one wait each)
# followed by a bare drain.
def _patched_drain_and_barrier(self, tick_clock, wait_clock):
    import bass_rust

    nop_inst = self.nc.sync.nop(nofuse=True, hint="pre_drain_wait_funnel")
    wait_clock.add_sem_waits(
        nop_inst.ins, ScopedClock({None: tick_clock.global_clock})
    )
    si = nop_inst.ins.sync_info
    waits = list(si.on_wait) if si else []
    if len(waits) > 1:
        nop_inst.ins.sync_info.on_wait = waits[:1]
        for w in waits[1:]:
            extra = self.nc.sync.nop(nofuse=True, hint="pre_drain_wait_funnel")
            extra.ins.sync_info = bass_rust.SyncInfo(on_wait=[w], on_update=[])
    self.nc.sync.drain()

    self.nc.all_engine_barrier()
    assert self.sems is not None
    popped = self.nc._tile_sem_poison_stack.pop()
    assert popped is self._sem_poison
    self.nc.clear_and_free_semaphores(list(self.sems.allocated().values()))
    self.nc.all_engine_barrier()


tile.TileContext._drain_and_barrier = _patched_drain_and_barrier


def _split_multi_waits(nc, max_waits=1):
    """This walrus build allows at most one sem wait per instruction. Hoist
    extra waits onto same-engine nops inserted immediately before."""
    import bass_rust

    n_split = 0
    for f in nc.m.functions:
        for bb in f.blocks:
            il = bb.instructions
            i = 0
            while i < len(il):
                inst = il[i]
                si = inst.sync_info
                if si is None or len(si.on_wait) <= max_waits:
                    i += 1
                    continue
                waits = list(si.on_wait)
                si.on_wait = waits[:max_waits]
                for k, w in enumerate(waits[max_waits:]):
                    nop = mybir.InstNoOp(
                        name=f"{inst.name}-wsplit{k}", ins=[], outs=[]
                    )
                    nop.engine = inst.engine
                    nop.sync_info = bass_rust.SyncInfo(on_wait=[w], on_update=[])
                    il.insert(i, nop)
                    i += 1
                n_split += 1
                i += 1
    return n_split
# ---------------------------------------------------------------------------


def _build_kernel(C, reps=1):
    """C: per-core expert token capacity (multiple of 128).

    reps>1 repeats the whole compute phase (DMAs included once) — used by
    test.py for slope-based timing that cancels per-call dispatch overhead.
    """
    CT = C // 128  # token tiles of gathered tokens

    nc = bass.Bass()
    # All inputs arrive pre-tiled by the host into [128, ...] layouts so each
    # tensor loads with one or two large DMAs (per-DMA issue overhead is
    # ~0.8us, and the first matmuls otherwise starve).
    x4_d = nc.dram_tensor("x4", [4 * 128, DC, 512], BF16, kind="ExternalInput")
    xg_d = nc.dram_tensor("xg", [128, DC * C], BF16, kind="ExternalInput")
    wg_d = nc.dram_tensor("wg", [128, CT], F32, kind="ExternalInput")
    w13_d = nc.dram_tensor("w13", [2 * 128, DC, 2048], BF16, kind="ExternalInput")
    w2_d = nc.dram_tensor("w2", [128, HC * DIM], BF16, kind="ExternalInput")
    s13_d = nc.dram_tensor("s13", [128, DC * 2 * SH], BF16, kind="ExternalInput")
    s2_d = nc.dram_tensor("s2", [128, SC * DIM], BF16, kind="ExternalInput")
    xo_d = nc.dram_tensor("xo", [128, DC * CO], BF16, kind="ExternalInput")
    wgo_d = nc.dram_tensor("wgo", [128, 1], F32, kind="ExternalInput")
    w13q_d = nc.dram_tensor("w13q", [128, DC * 2 * QH], BF16, kind="ExternalInput")
    w2q_d = nc.dram_tensor("w2q", [128, QC * DIM], BF16, kind="ExternalInput")
    yg_d = nc.dram_tensor("yg", [C, DIM], BF16, kind="ExternalOutput")
    ys_d = nc.dram_tensor("ys", [T, DIM], BF16, kind="ExternalOutput")
    yo_d = nc.dram_tensor("yo", [CO, DIM], BF16, kind="ExternalOutput")

    with tile.TileContext(nc) as tc, ExitStack() as ctx:
        persist = ctx.enter_context(tc.tile_pool(name="persist", bufs=1))
        silu_p = ctx.enter_context(tc.tile_pool(name="silu", bufs=3))
        yo_p = ctx.enter_context(tc.tile_pool(name="yo", bufs=6))
        h_ps = ctx.enter_context(tc.tile_pool(name="h_ps", bufs=4, space="PSUM"))
        y_ps = ctx.enter_context(tc.tile_pool(name="y_ps", bufs=4, space="PSUM"))

        # Persistent SBUF tensors (pre-transposed bf16 operands)
        xT4 = persist.tile([128, 4, DC, 512], BF16, tag="xT4")   # all tokens, quarter-major
        xgT = persist.tile([128, DC, C], BF16, tag="xgT")        # routed tokens
        w13T = persist.tile([128, 2, DC, 2048], BF16, tag="w13T")  # w1|w3 by hid-half
        w2T = persist.tile([128, HC, DIM], BF16, tag="w2T")      # [hid, d]
        s13T = persist.tile([128, DC, 2 * SH], BF16, tag="s13T")  # s1|s3 [d, hid_sl]
        s2T = persist.tile([128, SC, DIM], BF16, tag="s2T")
        wg = persist.tile([128, CT], F32, tag="wg")              # combine weight/token
        xoT = persist.tile([128, DC, CO], BF16, tag="xoT")       # overflow tokens
        w13qT = persist.tile([128, DC, 2 * QH], BF16, tag="w13qT")  # w1q|w3q [d, hid_q]
        w2qT = persist.tile([128, QC, DIM], BF16, tag="w2qT")
        wgo = persist.tile([128, 1], F32, tag="wgo")
        hT = persist.tile([128, HC, C], BF16, tag="hT")          # expert hidden
        shT = persist.tile([128, SC, T], BF16, tag="shT")        # shared hidden
        hoT = persist.tile([128, QC, CO], BF16, tag="hoT")       # overflow hidden
        ident = persist.tile([128, 128], F32, tag="ident")       # PE warm-up fodder

        # PE warm-up: the tensor engine ramps 0.65 -> 1.2 -> 2.4 GHz over
        # ~3us of continuous work. Burn the initial DMA-wait window on dummy
        # fp32 matmuls (no input deps) so the real chains start at full clock.
        make_identity(nc, ident)
        for wu in range(3):
            pw = h_ps.tile([128, 512], F32, tag="hps")
            for j in range(4):
                nc.tensor.matmul(
                    pw[:, 0:128], ident[:], ident[:],
                    start=(j == 0), stop=(j == 3),
                )

        # Issue order == consumption order; transfers drain FIFO through the
        # shared DMA engines, so x quarters land just ahead of the PE. The
        # first pieces are small so the first matmul chain starts ASAP.
        nc.sync.dma_start(s13T[:, :, 0:SH], s13_d[:, 0:DC * SH])
        nc.sync.dma_start(xT4[:, 0, :, 0:128], x4_d[0:128, :, 0:128])
        nc.sync.dma_start(s13T[:, :, SH:2 * SH], s13_d[:, DC * SH:DC * 2 * SH])
        nc.sync.dma_start(xT4[:, 0, :, 128:256], x4_d[0:128, :, 128:256])
        nc.sync.dma_start(xT4[:, 0, :, 256:512], x4_d[0:128, :, 256:512])
        for q in range(1, 4):
            nc.sync.dma_start(xT4[:, q, :, :], x4_d[q * 128:(q + 1) * 128, :, :])
        nc.sync.dma_start(xgT[:], xg_d[:])
        # w1/w3 pieces land in consumption order: hb 0-3's w1 then w3, etc.
        for g in range(2):
            gsl = slice(g * 128, (g + 1) * 128)
            for j0 in (0, 1024):
                nc.sync.dma_start(
                    w13T[:, g, :, j0:j0 + 512], w13_d[gsl, :, j0:j0 + 512]
                )
            for j0 in (512, 1536):
                nc.sync.dma_start(
                    w13T[:, g, :, j0:j0 + 512], w13_d[gsl, :, j0:j0 + 512]
                )
        nc.sync.dma_start(s2T[:], s2_d[:])
        nc.sync.dma_start(xoT[:], xo_d[:])
        nc.sync.dma_start(w13qT[:], w13q_d[:])
        nc.sync.dma_start(w2T[:], w2_d[:])
        nc.sync.dma_start(w2qT[:], w2q_d[:])
        nc.sync.dma_start(wg[:], wg_d[:])
        nc.sync.dma_start(wgo[:], wgo_d[:])

        def swiglu_h(lhs1, lhs3, rhs, hout, hb, fsl, fw):
            """lhs1/lhs3: dc -> [128,128] AP; rhs: dc -> [128,fw] AP."""
            p1 = h_ps.tile([128, 512], F32, tag="hps")
            for dc in range(DC):
                nc.tensor.matmul(
                    p1[:, :fw], lhs1(dc), rhs(dc),
                    start=(dc == 0), stop=(dc == DC - 1),
                )
            p3 = h_ps.tile([128, 512], F32, tag="hps")
            for dc in range(DC):
                nc.tensor.matmul(
                    p3[:, :fw], lhs3(dc), rhs(dc),
                    start=(dc == 0), stop=(dc == DC - 1),
                )
            sl = silu_p.tile([128, 512], BF16, tag="silu")
            nc.scalar.activation(sl[:, :fw], p1[:, :fw], AF.Silu)
            nc.vector.tensor_tensor(
                hout[:, hb, fsl], sl[:, :fw], p3[:, :fw], op=OP.mult
            )

        fblocks = []
        fo = 0
        while fo < C:
            fw = min(512, C - fo)
            fblocks.append((slice(fo, fo + fw), fw))
            fo += fw

        for _rep in range(reps):
            # --- shared-expert h-stage: shT[hid_sl, t] over all tokens.
            # Quarter-outer so PE consumption tracks the x4 DMA arrivals; the
            # first quarter runs as two 256-wide chains to match its split DMA.
            for q in range(4):
                subs = [(0, 128), (128, 128), (256, 256)] if q == 0 else [(0, 512)]
                for so, sw in subs:
                    for hb in range(SC):
                        swiglu_h(
                            lambda dc, hb=hb: s13T[:, dc, hb * 128:(hb + 1) * 128],
                            lambda dc, hb=hb: s13T[:, dc, SH + hb * 128:SH + (hb + 1) * 128],
                            lambda dc, q=q, so=so, sw=sw: xT4[:, q, dc, so:so + sw],
                            shT, hb, slice(q * 512 + so, q * 512 + so + sw), sw,
                        )

            # --- expert h-stage: hT[hid, t] over routed tokens
            for hb in range(HC):
                g, r = divmod(hb, 8)
                for fsl, fw in fblocks:
                    swiglu_h(
                        lambda dc, g=g, r=r: w13T[:, g, dc, r * 128:(r + 1) * 128],
                        lambda dc, g=g, r=r: w13T[:, g, dc, 1024 + r * 128:1024 + (r + 1) * 128],
                        lambda dc, fsl=fsl: xgT[:, dc, fsl],
                        hT, hb, fsl, fw,
                    )

            # --- overflow h-stage: quarter-HID FFN for the overflow tokens
            for hq in range(QC):
                swiglu_h(
                    lambda dc, hq=hq: w13qT[:, dc, hq * 128:(hq + 1) * 128],
                    lambda dc, hq=hq: w13qT[:, dc, QH + hq * 128:QH + (hq + 1) * 128],
                    lambda dc: xoT[:, dc, :],
                    hoT, hq, slice(0, CO), CO,
                )

            # --- shared mm2: ys[t, d] = shT.T @ s2T, per 128-token tile.
            # Runs between the h stages and expert-mm2: its matmuls only need
            # shT (done early), so they fill the PE bubble while the last
            # expert hT tiles drain through silu/mult. PSUM->bf16 copies split
            # between Act and DVE, output DMAs split between SP and Pool, so
            # no single engine gates the PE.
            for tb in range(TT):
                tsl = slice(tb * 128, (tb + 1) * 128)
                yso = yo_p.tile([128, DIM], BF16, tag="yso")
                for dh in range(2):
                    dsl = slice(dh * 384, (dh + 1) * 384)
                    ps = y_ps.tile([128, 384], F32, tag="y")
                    for sc in range(SC):
                        nc.tensor.matmul(
                            ps[:], shT[:, sc, tsl], s2T[:, sc, dsl],
                            start=(sc == 0), stop=(sc == SC - 1),
                        )
                    if dh == 0:
                        nc.scalar.copy(yso[:, dsl], ps[:])
                    else:
                        nc.vector.tensor_copy(yso[:, dsl], ps[:])
                eng = nc.sync if tb % 2 == 0 else nc.gpsimd
                eng.dma_start(ys_d[tb * 128:(tb + 1) * 128, :], yso[:])

            # --- overflow mm2: yo[t, d] = (hoT.T @ w2qT) * wgo
            yoo = yo_p.tile([128, DIM], BF16, tag="ygo")
            for dh in range(2):
                dsl = slice(dh * 384, (dh + 1) * 384)
                po = y_ps.tile([128, 384], F32, tag="y")
                for hq in range(QC):
                    nc.tensor.matmul(
                        po[0:CO, :], hoT[:, hq, :], w2qT[:, hq, dsl],
                        start=(hq == 0), stop=(hq == QC - 1),
                    )
                nc.vector.tensor_scalar(
                    yoo[0:CO, dsl], po[0:CO, :], wgo[0:CO, 0:1], None, op0=OP.mult
                )
            nc.gpsimd.dma_start(yo_d[:], yoo[0:CO, :])

            # --- expert mm2: yg[t, d] = (hT.T @ w2T) * wg, per 128-token tile
            for tb in range(CT):
                tsl = slice(tb * 128, (tb + 1) * 128)
                yo = yo_p.tile([128, DIM], BF16, tag="yo")
                for dh in range(2):
                    dsl = slice(dh * 384, (dh + 1) * 384)
                    pe = y_ps.tile([128, 384], F32, tag="y")
                    for hb in range(HC):
                        nc.tensor.matmul(
                            pe[:], hT[:, hb, tsl], w2T[:, hb, dsl],
                            start=(hb == 0), stop=(hb == HC - 1),
                        )
                    nc.vector.tensor_scalar(
                        yo[:, dsl], pe[:], wg[:, tb:tb + 1], None, op0=OP.mult
                    )
                nc.sync.dma_start(yg_d[tb * 128:(tb + 1) * 128, :], yo[:])

    _split_multi_waits(nc)
    try:
        _CACHE["makespan_ns"] = max(e[2] for e in tc._perfetto_entries)
    except Exception:
        _CACHE["makespan_ns"] = None
    return nc


_CACHE = {}


def _route(x2, gate_w):
    """Host gate: softmax + top-2 + normalize. Returns per-expert token
    lists and combine weights."""
    logits = x2 @ gate_w.T                       # [T, E] f32
    logits -= logits.max(1, keepdims=True)
    p = np.exp(logits)
    p /= p.sum(1, keepdims=True)
    tix = np.arange(p.shape[0])
    i1 = p.argmax(1)
    pm = p.copy()
    pm[tix, i1] = -np.inf
    i2 = pm.argmax(1)
    p1 = p[tix, i1]
    p2 = p[tix, i2]
    s = p1 + p2 + 1e-20
    w1n = p1 / s
    w2n = p2 / s
    sels, wgts = [], []
    for c in range(E):
        sel = np.flatnonzero((i1 == c) | (i2 == c))
        w_sel = np.where(i1[sel] == c, w1n[sel], w2n[sel]).astype(np.float32)
        sels.append(sel)
        wgts.append(w_sel)
    return sels, wgts


def _tile_cols(aT, nchunks):
    # aT: [nchunks*128, M] -> [128, nchunks*M] with [p, c*M+m] = aT[c*128+p, m]
    M = aT.shape[1]
    return np.ascontiguousarray(
        aT.reshape(nchunks, 128, M).transpose(1, 0, 2).reshape(128, nchunks * M)
    )


def kernel(x, gate_w, w1, w2, w3, ws1, ws2, ws3):
    x = np.asarray(x, dtype=np.float32)
    gate_w = np.ascontiguousarray(np.asarray(gate_w, dtype=np.float32))
    w1 = np.asarray(w1, dtype=np.float32)
    w2 = np.asarray(w2, dtype=np.float32)
    w3 = np.asarray(w3, dtype=np.float32)
    ws1 = np.asarray(ws1, dtype=np.float32)
    ws2 = np.asarray(ws2, dtype=np.float32)
    ws3 = np.asarray(ws3, dtype=np.float32)

    B, S, D = x.shape
    x2 = np.ascontiguousarray(x.reshape(-1, D))

    sels, wgts = _route(x2, gate_w)

    # Capacity plan: C=512 + one 128-token quarter-HID overflow slot per core
    # handles up to 2 experts exceeding C by up to CO tokens each. Otherwise
    # fall back to a plain capacity-padded build (correct for any routing).
    C = 512
    over = [c for c in range(E) if len(sels[c]) > C]
    if len(over) > 2 or any(len(sels[c]) > C + CO for c in over):
        C = -(-max(len(s) for s in sels) // 128) * 128
        over = []

    if _CACHE.get("C") != C:
        _CACHE["C"] = C
        _CACHE["nc"] = _build_kernel(C)
    nc = _CACHE["nc"]
    CT = C // 128

    xb = x2.astype(bfloat16)
    # x4[q*128+p, dc*512+j] = x[q*512+j, dc*128+p]
    x4 = np.ascontiguousarray(
        xb.reshape(4, 512, DC, 128).transpose(0, 3, 2, 1).reshape(4 * 128, DC, 512)
    )

    # overflow slot assignment: expert over[k] -> cores 4k..4k+3, quarter q
    ovf_sel = {}
    for k, e in enumerate(over):
        ovf_sel[e] = sels[e][C:]

    in_maps = []
    for c in range(N_CORES):
        sel = sels[c][:C]
        n = len(sel)
        xgT = np.zeros((D, C), dtype=bfloat16)
        xgT[:, :n] = x2[sel].T
        wgv = np.zeros((C,), dtype=np.float32)
        wgv[:n] = wgts[c][:n]
        sh = slice(c * SH, (c + 1) * SH)
        # w13[g*128+p, dc*2048+j]: j<1024 -> w1T[dc*128+p, g*1024+j], else w3T
        w1r = w1[c].T.astype(bfloat16).reshape(DC, 128, 2, 1024).transpose(2, 1, 0, 3)
        w3r = w3[c].T.astype(bfloat16).reshape(DC, 128, 2, 1024).transpose(2, 1, 0, 3)
        w13 = np.ascontiguousarray(
            np.concatenate([w1r, w3r], axis=3).reshape(2 * 128, DC, 2048)
        )
        # s13 halves: cols [0:DC*SH] = s1 (dc-major), [DC*SH:] = s3 — matches
        # the two split DMAs whose dst is s13T[:, :, 0:SH] / [:, :, SH:2*SH].
        s1r = ws1[sh].T.astype(bfloat16).reshape(DC, 128, SH).transpose(1, 0, 2)
        s3r = ws3[sh].T.astype(bfloat16).reshape(DC, 128, SH).transpose(1, 0, 2)
        s13 = np.ascontiguousarray(np.concatenate(
            [s1r.reshape(128, DC * SH), s3r.reshape(128, DC * SH)], axis=1
        ))

        # overflow slot inputs for this core
        xoT = np.zeros((D, CO), dtype=bfloat16)
        wgo = np.zeros((128,), dtype=np.float32)
        w13q = np.zeros((D, 2 * QH), dtype=bfloat16)
        w2q = np.zeros((QH, DIM), dtype=bfloat16)
        k, q = divmod(c, 4)
        if k < len(over):
            e = over[k]
            osel = ovf_sel[e]
            no = len(osel)
            xoT[:, :no] = x2[osel].T
            wgo[:no] = wgts[e][C:]
            h0 = q * QH
            w13q[:, :QH] = w1[e].T[:, h0:h0 + QH].astype(bfloat16)
            w13q[:, QH:] = w3[e].T[:, h0:h0 + QH].astype(bfloat16)
            w2q[:, :] = w2[e].T[h0:h0 + QH, :].astype(bfloat16)

        in_maps.append({
            "x4": x4,
            "xg": _tile_cols(xgT, DC),
            "wg": np.ascontiguousarray(wgv.reshape(CT, 128).T),
            "w13": w13,
            "w2": _tile_cols(w2[c].T.astype(bfloat16), HC),
            "s13": s13,
            "s2": _tile_cols(ws2[:, sh].T.astype(bfloat16), SC),
            "xo": _tile_cols(xoT, DC),
            "wgo": np.ascontiguousarray(wgo.reshape(1, 128).T),
            "w13q": _tile_cols(w13q, DC),
            "w2q": _tile_cols(w2q, QC),
        })

    _CACHE["last_in_maps"] = in_maps
    res = run_bass_kernel_spmd(nc, in_maps, list(range(N_CORES)))
    y = np.zeros((T, D), dtype=np.float32)
    for c in range(N_CORES):
        sel = sels[c][:C]
        y[sel] += np.asarray(res.results[c]["yg"][:len(sel)], dtype=np.float32)
        y += np.asarray(res.results[c]["ys"], dtype=np.float32)
        k = c // 4
        if k < len(over):
            osel = ovf_sel[over[k]]
            y[osel] += np.asarray(res.results[c]["yo"][:len(osel)], dtype=np.float32)
    return y.reshape(B, S, D)
